# revision 33
# baseline (speedup 1.0000x reference)
"""BitLinear forward on 8 Trainium2 NeuronCores.

y = activation_quant(x) @ weight_quant(w).T

Strategy (column-parallel / tensor-parallel on out_features):
  - each core holds the full x and a 1/8 shard of w along out_features
  - weight scale (mean|w|) computed on-device with a 1-scalar AllReduce
  - activation quant: per-token int8 values, stored exactly in bf16
  - weight quant: ternary {-1,0,1}, stored exactly in bf16 for the DMA
    transpose, then converted to fp8e4 (still exact) for SBUF residency
  - matmul: bf16 stationary (x^T tile) x fp8 moving (w^T) -> exact fp32
    integer accumulation in PSUM at full PE rate
  - per-token output scale (1/act_scale * mean|w|) applied during PSUM
    drain; the weight-scale factor is folded into the drain scale so the
    activation-quant stream never waits on the AllReduce
  - host concatenates the 8 output shards along out_features

Engine/queue assignment (chosen so the x-quant stream never sits behind
matmul-paced work in any in-order queue):
  SP    all x-side DMA: x loads, xq stores, xq transpose-reads, and the
        phase-C wq stores + wqt transpose-reads (SP is x-idle then)
  DVE   all x-quant compute (absmax reduce, fused scale+round via
        per-partition tensor_scalar), w rounds/clips, fp8 converts
  ACT   w loads + w scale-copies, then only drain-side work (per-token
        drain scale, PSUM drains) which is matmul-paced by nature
  Pool  the 1-scalar AllReduce and y stores

Phases: (A) w pass-1 |w| sums stream while the first 8 token tiles
quantize; (B) AllReduce, with 4 more token tiles filling the wait;
(C) w pass-2 burst (requant, DRAM-transpose round-trip, fp8 convert);
(D) matmul stream, x-quant running free ~2 blocks ahead.
"""

import numpy as np
from contextlib import ExitStack

import concourse.bass as bass
import concourse.tile as tile
from concourse import bacc, mybir
from concourse import masks
from concourse.bass import ts, ds
from concourse.bass_utils import run_bass_kernel_spmd

F32 = mybir.dt.float32
BF16 = mybir.dt.bfloat16
FP8 = mybir.dt.float8e4
AX = mybir.AxisListType
OP = mybir.AluOpType
ACTF = mybir.ActivationFunctionType

# 1.5 * 2^23: adding then subtracting performs round-to-nearest-even at
# integer granularity in fp32 (matches jnp.round for |v| <= 2^21).
MAGIC = 12582912.0
EPS = 1e-5
P = 128


class _Builder:
    def __init__(self, nc, M, K, N, count_total, n_cores,
                 skip_quant=False, skip_matmul=False, wqt_dtype=FP8):
        self.nc = nc
        self.M, self.K, self.N = M, K, N
        self.count_total = count_total
        self.n_cores = n_cores
        self.skip_quant = skip_quant
        self.skip_matmul = skip_matmul
        self.wqt_dtype = wqt_dtype
        self.KT = K // P          # 32 k-slices
        self.MT = M // P          # 64 token tiles
        self.NCH = 512
        self.NT = N // self.NCH   # 4 psum chunks
        self.MB = 512             # token rows per transpose block
        self.MSUB = self.MB // P  # 4 tiles per block
        self.MBT = M // self.MB   # 16 blocks
        self.XH = K // 2          # half-row DMA split
        self.WT = N // P          # 16 weight tiles
        # x-quant emission runs this many tiles ahead of the matmul stream
        self.QAHEAD = 8
        self._xpend = {}
        self._next_load = 0
        self._next_comp = 0

    def build(self, reps=1):
        nc = self.nc
        self.x = nc.dram_tensor("x", [self.M, self.K], F32,
                                kind="ExternalInput").ap()
        self.w = nc.dram_tensor("w", [self.N, self.K], F32,
                                kind="ExternalInput").ap()
        self.y = nc.dram_tensor("y", [self.M, self.N], F32,
                                kind="ExternalOutput").ap()

        with tile.TileContext(nc) as tc, ExitStack() as ctx:
            self.tc = tc
            dram = ctx.enter_context(
                tc.tile_pool(name="dram", bufs=1, space="DRAM"))
            self.xq_dram = dram.tile([self.M, self.K], BF16)
            self.wq_dram = dram.tile([self.N, self.K], BF16)
            self.cc_in = dram.tile([1, 1], F32)
            self.cc_out = dram.tile([1, 1], F32)

            const = ctx.enter_context(tc.tile_pool(name="const", bufs=1))
            self.onesf = const.tile([P, P], F32)
            nc.vector.memset(self.onesf[:], 1.0)
            self.ident = const.tile([P, P], BF16)
            masks.make_identity(nc, self.ident[:])
            self.alphas = const.tile([P, self.MT], F32)   # 1/act_scale
            self.alphaf = const.tile([P, self.MT], F32)   # * s_w
            self.wsums = const.tile([P, self.WT], F32)
            self.s_w_eff = const.tile([P, 1], F32)
            self.s_inv = const.tile([P, 1], F32)

            self.stat = ctx.enter_context(tc.tile_pool(name="stat", bufs=8))

            if self.skip_quant:
                nc.vector.memset(self.alphaf[:], 1.0)
                seed = const.tile([P, 64], BF16)
                nc.vector.memset(seed[:], 1.0)
                nc.gpsimd.dma_start(self.xq_dram[0:P, 0:64], seed[:])
                nc.gpsimd.dma_start(self.wq_dram[0:P, 0:64], seed[:])

            for _ in range(reps):
                self._xpend = {}
                self._next_load = 0
                self._next_comp = 0
                self.emit_once(ctx)
        return nc

    def emit_once(self, ctx):
        with ExitStack() as mctx:
            tc = self.tc
            if not self.skip_matmul:
                self.wqt_pool = mctx.enter_context(
                    tc.tile_pool(name="wqt", bufs=1))
                self.wqt = self.wqt_pool.tile(
                    [P, self.KT, self.N], self.wqt_dtype)
            if not self.skip_quant:
                self.xin = mctx.enter_context(
                    tc.tile_pool(name="x_in", bufs=3))
                self.xqs = mctx.enter_context(
                    tc.tile_pool(name="xq_st", bufs=2))

            if self.skip_quant:
                with ExitStack() as wctx:
                    self.wtrs = wctx.enter_context(
                        tc.tile_pool(name="wtrs", bufs=2))
                    self.emit_wqt()
                self.open_matmul_pools(mctx)
                for mb in range(self.MBT):
                    xqtt = self.emit_block_transposes(mb)
                    for ms in range(self.MSUB):
                        self.emit_tile_matmul(
                            mb * self.MSUB + ms, xqtt, ms, seeded=True)
            elif self.skip_matmul:
                with ExitStack() as wctx:
                    self.win = wctx.enter_context(
                        tc.tile_pool(name="w_in", bufs=2))
                    self.wqs = wctx.enter_context(
                        tc.tile_pool(name="wq_st", bufs=2))
                    scr = wctx.enter_context(
                        tc.tile_pool(name="wscr", bufs=2))
                    for t in range(self.WT):
                        self.emit_wreduce(t)
                    with ExitStack() as sctx:
                        self.pss = sctx.enter_context(
                            tc.tile_pool(name="pssmall", bufs=1,
                                         space="PSUM"))
                        self.emit_scale_pre()
                        self.emit_scale_post()
                    self.psc = wctx.enter_context(
                        tc.tile_pool(name="psc", bufs=8, space="PSUM"))
                    self.emit_wpass2(scratch=scr)
                for _ in range(self.MT):
                    self.emit_xq_step()
            else:
                self.emit_main(mctx)

    def open_matmul_pools(self, mctx):
        tc = self.tc
        self.xqt = mctx.enter_context(tc.tile_pool(name="xqt", bufs=2))
        self.pspool = mctx.enter_context(
            tc.tile_pool(name="ps", bufs=2, space="PSUM"))
        self.outp = mctx.enter_context(tc.tile_pool(name="out", bufs=2))

    # ---------------- weight pass 1: |w| partial sums ---------------------
    def emit_wreduce(self, t):
        nc = self.nc
        wt = self.win.tile([P, self.K], F32, tag="w", name="wt")
        for h in range(2):
            nc.scalar.dma_start(
                wt[:, ds(h * self.XH, self.XH)],
                self.w[ts(t, P), ds(h * self.XH, self.XH)])
        nc.vector.tensor_reduce(
            self.wsums[:, ds(t, 1)], wt[:], axis=AX.X, op=OP.add,
            apply_absolute_value=True)

    # ---------------- global scale + AllReduce ----------------------------
    def emit_scale_pre(self):
        nc = self.nc
        wpart = self.stat.tile([P, 1], F32, tag="wpart", name="wpart")
        nc.vector.tensor_reduce(
            wpart[:], self.wsums[:], axis=AX.X, op=OP.add)
        ps1 = self.pss.tile([1, 1], F32, tag="small", name="ps1")
        # ones[128,1].T @ wpart[128,1] -> total |w| sum on this core
        nc.tensor.matmul(
            ps1[:], self.onesf[:, 0:1], wpart[:], start=True, stop=True)
        ccs = self.stat.tile([1, 1], F32, tag="ccs", name="ccs")
        nc.vector.tensor_copy(ccs[:], ps1[:])
        nc.sync.dma_start(self.cc_in[:], ccs[:])
        if self.n_cores > 1:
            nc.gpsimd.collective_compute(
                "AllReduce", OP.add,
                replica_groups=[list(range(self.n_cores))],
                ins=[self.cc_in.opt()], outs=[self.cc_out.opt()])
        else:
            nc.gpsimd.dma_start(self.cc_out[:], self.cc_in[:])

    def emit_scale_post(self):
        nc = self.nc
        ccb = self.stat.tile([1, 1], F32, tag="ccb", name="ccb")
        nc.sync.dma_start(ccb[:], self.cc_out[:])
        ps2 = self.pss.tile([P, 1], F32, tag="small", name="ps2")
        # ones[1,128].T @ val[1,1] -> broadcast scalar to 128 partitions
        nc.tensor.matmul(
            ps2[:], self.onesf[0:1, :], ccb[:], start=True, stop=True)
        # mean = sum/count; count is a power of two so multiply is exact
        assert self.count_total & (self.count_total - 1) == 0
        s_w = self.stat.tile([P, 1], F32, tag="s_w", name="s_w")
        nc.vector.tensor_scalar(
            s_w[:], ps2[:], 1.0 / float(self.count_total), EPS,
            OP.mult, OP.max)
        nc.vector.reciprocal(self.s_inv[:], s_w[:])
        nc.vector.reciprocal(self.s_w_eff[:], self.s_inv[:])

    # ---------------- weight pass 2: requant + PE transpose + fp8 ---------
    def _wreload(self, t):
        nc = self.nc
        wt = self.win.tile([P, self.K], F32, tag="w", name="wt")
        for h in range(2):
            nc.scalar.dma_start(
                wt[:, ds(h * self.XH, self.XH)],
                self.w[ts(t, P), ds(h * self.XH, self.XH)])
        return wt

    def emit_wpass2(self, scratch=None, x_interleave=0):
        """Requant each w tile and transpose it on the PE straight into
        the fp8 wqt resident tensor: no DRAM round-trip and no all-tiles
        barrier -- each tile pipelines independently. The PSUM->wqt copy
        for a tile is deferred until its psc slot is about to be reused,
        so the ACT queue never serializes consecutive tiles' chains.
        x_interleave > 0 emits one x-quant step after every 2nd w tile
        (for the first 2*x_interleave w tiles) to keep the token-tile
        ring filling while this burst owns the DVE."""
        nc = self.nc
        PSC_BUFS = 8
        pending = []
        pend = self._wreload(0)
        for t in range(self.WT):
            wt = pend
            pend = self._wreload(t + 1) if t + 1 < self.WT else None
            if x_interleave > 0 and t % 2 == 1:
                self.emit_xq_step()
                x_interleave -= 1
            # fused quant, all on DVE: round(w*s_inv) then clip to [-1,1]
            nc.vector.tensor_scalar(
                wt[:], wt[:], self.s_inv[:, 0:1], MAGIC, OP.mult, OP.add)
            nc.vector.tensor_scalar(
                wt[:], wt[:], -MAGIC, 1.0, OP.add, OP.min)
            wq = self.wqs.tile([P, self.K], BF16, tag="wq", name="wq")
            nc.vector.tensor_scalar(
                wq[:], wt[:], -1.0, None, OP.max)
            for kk in range(self.KT // 4):
                # flush the copy whose psc slot this allocation reuses, so
                # the reader is emitted before the slot's next writer
                if len(pending) >= PSC_BUFS:
                    pst, dst = pending.pop(0)
                    nc.scalar.activation(dst, pst[:, :, :], ACTF.Copy)
                pst = self.psc.tile([P, 4, P], BF16, tag="psc", name="pst")
                for j in range(4):
                    nc.tensor.transpose(
                        pst[:, j, :], wq[:, ds((4 * kk + j) * P, P)],
                        self.ident[:])
                if scratch is not None:
                    dst = scratch.tile([P, 4, P], self.wqt_dtype,
                                       tag="scr", name="scr")[:, :, :]
                else:
                    dst = self.wqt[:, ds(4 * kk, 4), ds(t * P, P)]
                pending.append((pst, dst))
        for pst, dst in pending:
            nc.scalar.activation(dst, pst[:, :, :], ACTF.Copy)

    def emit_wqt(self):
        # transpose-read each 128-col slice of wq_dram and convert to fp8
        # (only used by the skip_quant timing-ablation path)
        nc = self.nc
        for k in range(self.KT):
            wtr = self.wtrs.tile([P, self.N], BF16, tag="wtr", name="wtr")
            nc.sync.dma_start_transpose(
                wtr[:], self.wq_dram[:, ds(k * P, P)])
            nc.vector.tensor_copy(self.wqt[:, k, :], wtr[:])

    # ---------------- activation quant, software-pipelined ----------------
    def emit_xload(self, mt):
        nc = self.nc
        xt = self.xin.tile([P, self.K], F32, tag="x", name="xt")
        for h in range(2):
            nc.sync.dma_start(
                xt[:, ds(h * self.XH, self.XH)],
                self.x[ts(mt, P), ds(h * self.XH, self.XH)])
        self._xpend[mt] = xt

    def emit_xcompute(self, mt):
        nc = self.nc
        stat = self.stat
        xt = self._xpend.pop(mt)
        am = stat.tile([P, 1], F32, tag="am", name="am")
        nc.vector.tensor_reduce(
            am[:], xt[:], axis=AX.X, op=OP.max, apply_absolute_value=True)
        ame = stat.tile([P, 1], F32, tag="ame", name="ame")
        nc.vector.tensor_scalar(ame[:], am[:], EPS, None, OP.max)
        rec = stat.tile([P, 1], F32, tag="rec", name="rec")
        nc.vector.reciprocal(rec[:], ame[:])
        scale = stat.tile([P, 1], F32, tag="scale", name="scale")
        nc.vector.tensor_scalar(scale[:], rec[:], 127.0, None, OP.mult)
        # alphas[:, mt] = 1/scale (s_w factor folded in at drain time)
        nc.vector.reciprocal(self.alphas[:, ds(mt, 1)], scale[:])
        # fused quant: (x * scale + MAGIC) - MAGIC, rounded to int in bf16
        nc.vector.tensor_scalar(
            xt[:], xt[:], scale[:, 0:1], MAGIC, OP.mult, OP.add)
        xq = self.xqs.tile([P, self.K], BF16, tag="xq", name="xq")
        nc.vector.tensor_scalar(xq[:], xt[:], -MAGIC, None, OP.add)
        nc.sync.dma_start(self.xq_dram[ts(mt, P), :], xq[:])

    def emit_xq_step(self):
        # one tile of quant progress: prefetch next loads, compute current
        if self._next_load < self.MT:
            self.emit_xload(self._next_load)
            self._next_load += 1
        if self._next_comp < self._next_load:
            self.emit_xcompute(self._next_comp)
            self._next_comp += 1

    # ---------------- matmul stream ---------------------------------------
    def emit_block_transposes(self, mb):
        nc = self.nc
        xqtt = self.xqt.tile([P, self.KT, self.MB], BF16, tag="xqt",
                             name="xqtt")
        for k in range(self.KT):
            nc.sync.dma_start_transpose(
                xqtt[:, k, :],
                self.xq_dram[ds(mb * self.MB, self.MB), ds(k * P, P)])
        return xqtt

    def _drain(self, mt, nn, psum):
        nc = self.nc
        ot = self.outp.tile([P, self.NCH], F32, tag="o", name="ot")
        nc.scalar.activation(
            ot[:], psum[:], ACTF.Copy, scale=self.alphaf[:, ds(mt, 1)])
        nc.gpsimd.dma_start(
            self.y[ts(mt, P), ds(nn * self.NCH, self.NCH)], ot[:])

    def emit_tile_matmul(self, mt, xqtt, ms, seeded=False, nn_outer=False):
        nc = self.nc
        if not seeded:
            # fold s_w into the per-token drain scale
            nc.scalar.activation(
                self.alphaf[:, ds(mt, 1)], self.alphas[:, ds(mt, 1)],
                ACTF.Copy, scale=self.s_w_eff[:, 0:1])
        if nn_outer:
            # consume wqt in n-column order: lets the first tiles' matmuls
            # start while weight pass-2 is still producing wqt chunks
            for nn in range(self.NT):
                psum = self.pspool.tile(
                    [P, self.NCH], F32, tag=f"ps{nn}", name=f"ps{nn}")
                for k in range(self.KT):
                    nc.tensor.matmul(
                        psum[:], xqtt[:, k, ds(ms * P, P)],
                        self.wqt[:, k, ds(nn * self.NCH, self.NCH)],
                        start=(k == 0), stop=(k == self.KT - 1))
                self._drain(mt, nn, psum)
            return
        psums = [
            self.pspool.tile([P, self.NCH], F32, tag=f"ps{nn}", name=f"ps{nn}")
            for nn in range(self.NT)
        ]
        for k in range(self.KT):
            lhs = xqtt[:, k, ds(ms * P, P)]
            first, last = (k == 0), (k == self.KT - 1)
            for nn in range(self.NT):
                nc.tensor.matmul(
                    psums[nn][:], lhs,
                    self.wqt[:, k, ds(nn * self.NCH, self.NCH)],
                    start=first, stop=last)
        for nn in range(self.NT):
            self._drain(mt, nn, psums[nn])

    # ---------------- full pipeline ---------------------------------------
    def emit_main(self, mctx):
        tc = self.tc
        with ExitStack() as wctx:
            self.win = wctx.enter_context(tc.tile_pool(name="w_in", bufs=2))
            self.wqs = wctx.enter_context(tc.tile_pool(name="wq_st", bufs=3))
            # Phase A: w pass-1 gets DMA priority (the AllReduce gates
            # everything w-side downstream).
            for t in range(self.WT):
                self.emit_wreduce(t)
            # Phase B: AllReduce; 4 token tiles quantize during the wait.
            with ExitStack() as sctx:
                self.pss = sctx.enter_context(
                    tc.tile_pool(name="pssmall", bufs=1, space="PSUM"))
                self.emit_scale_pre()
                for _ in range(4):
                    self.emit_xq_step()
                self.emit_scale_post()
            # Phase C: w pass-2 burst (requant + PE transpose + fp8), with
            # 6 more token tiles interleaved so the ring keeps filling.
            self.psc = wctx.enter_context(
                tc.tile_pool(name="psc", bufs=8, space="PSUM"))
            self.emit_wpass2(x_interleave=6)
        # Phase D: matmul stream. Prologue: the first 2 blocks run
        # nn-major (all 8 tiles' nn-chunk 0, then chunk 1, ...) so the PE
        # consumes wqt n-column groups as weight pass-2 produces them.
        self.open_matmul_pools(mctx)
        xb = [self.emit_block_transposes(0), self.emit_block_transposes(1)]
        for nn in range(self.NT):
            for mt in range(2 * self.MSUB):
                mb, ms = divmod(mt, self.MSUB)
                if nn == 0:
                    nc = self.nc
                    nc.scalar.activation(
                        self.alphaf[:, ds(mt, 1)], self.alphas[:, ds(mt, 1)],
                        ACTF.Copy, scale=self.s_w_eff[:, 0:1])
                psum = self.pspool.tile(
                    [P, self.NCH], F32, tag=f"ps{nn}", name=f"ps{nn}")
                for k in range(self.KT):
                    self.nc.tensor.matmul(
                        psum[:], xb[mb][:, k, ds(ms * P, P)],
                        self.wqt[:, k, ds(nn * self.NCH, self.NCH)],
                        start=(k == 0), stop=(k == self.KT - 1))
                self._drain(mt, nn, psum)
        for mb in range(2, self.MBT):
            # at mb=2 pre-emit 3 blocks of quant so the SP load stream is
            # not frozen behind T(b2), which waits for the prologue's ring
            # slot; afterwards stay QAHEAD tiles ahead of consumption
            want = min(self.MT, max(24, mb * self.MSUB + self.QAHEAD))
            while self._next_comp < want:
                self.emit_xq_step()
            xqtt = self.emit_block_transposes(mb)
            for ms in range(self.MSUB):
                self.emit_tile_matmul(mb * self.MSUB + ms, xqtt, ms)


def build_bitlinear(nc, M, K, N, count_total, n_cores, reps=1,
                    skip_quant=False, skip_matmul=False, wqt_dtype=FP8):
    return _Builder(nc, M, K, N, count_total, n_cores,
                    skip_quant=skip_quant, skip_matmul=skip_matmul,
                    wqt_dtype=wqt_dtype).build(reps)


def dedupe_ldweights(nc):
    """Drop InstLdweights that reload the exact weights already resident in
    the PE array. The tile pipeline splits every bf16 matmul into
    LDWEIGHTS+MATMUL(ldweights=False); consecutive matmuls sharing one
    stationary operand (the nn loop) therefore reload it redundantly.
    The stationary operand persists across non-self-loading matmuls, so a
    sync-free LDW identical to the previous one is a no-op. Run after
    nc.compile() so all semaphore passes have finalized sync_info."""
    removed = 0
    for fn in nc.m.functions:
        for blk in fn.blocks:
            last_sig = None
            keep = []
            for inst in blk.instructions:
                if isinstance(inst, mybir.InstLdweights):
                    a = inst.ins[0]
                    sig = (getattr(a, "memref", None), a.offset, str(a.ap),
                           str(a.dtype), str(inst.perf_mode),
                           str(inst.tile_position), str(inst.is_transpose))
                    si = inst.sync_info
                    clean = si is None or (not si.on_wait and not si.on_update)
                    if sig == last_sig and clean and sig[0] is not None:
                        removed += 1
                        continue
                    last_sig = sig
                elif isinstance(inst, mybir.InstMatmult):
                    if inst.ldweights is not False:
                        last_sig = None  # self-loading matmul clobbers weights
                elif getattr(inst, "engine", None) == mybir.EngineType.PE:
                    if inst.is_executable():
                        last_sig = None  # unknown PE instruction: be safe
                keep.append(inst)
            if len(keep) != len(blk.instructions):
                blk.instructions = keep
    return removed


# ----------------------------------------------------------------------------
# Host-side entry point
# ----------------------------------------------------------------------------

_FULL = dict(B=4, S=2048, K=4096, N_TOTAL=16384, N_CORES=8)
_CACHE = {}


def _make_nc(reps=1, skip_quant=False, skip_matmul=False):
    cfg = _FULL
    M = cfg["B"] * cfg["S"]
    n_shard = cfg["N_TOTAL"] // cfg["N_CORES"]
    nc = bacc.Bacc(
        "TRN2",
        target_bir_lowering=False,
        debug=False,
        num_devices=cfg["N_CORES"],
    )
    build_bitlinear(
        nc, M=M, K=cfg["K"], N=n_shard,
        count_total=cfg["N_TOTAL"] * cfg["K"],
        n_cores=cfg["N_CORES"],
        reps=reps, skip_quant=skip_quant, skip_matmul=skip_matmul,
    )
    nc.compile()
    dedupe_ldweights(nc)
    from concourse.bass_interp import get_hw_module
    nc.m = get_hw_module(nc.m)
    return nc


def _get_compiled():
    if "nc" not in _CACHE:
        _CACHE["nc"] = _make_nc()
    return _CACHE["nc"]


def kernel(x: np.ndarray, weight: np.ndarray, _trace: bool = False):
    cfg = _FULL
    M = cfg["B"] * cfg["S"]
    n_shard = cfg["N_TOTAL"] // cfg["N_CORES"]
    nc = _get_compiled()

    x2 = np.ascontiguousarray(np.asarray(x, dtype=np.float32).reshape(M, cfg["K"]))
    wf = np.asarray(weight, dtype=np.float32)
    in_maps = [
        {"x": x2, "w": np.ascontiguousarray(wf[i * n_shard:(i + 1) * n_shard])}
        for i in range(cfg["N_CORES"])
    ]
    res = run_bass_kernel_spmd(
        nc, in_maps, list(range(cfg["N_CORES"])), trace=_trace)
    _CACHE["last_result"] = res
    yfull = np.concatenate(
        [res.results[i]["y"] for i in range(cfg["N_CORES"])], axis=1)
    return yfull.reshape(cfg["B"], cfg["S"], cfg["N_TOTAL"])


# revision 43
# speedup vs baseline: 1.0101x; 1.0101x over previous
"""BitLinear forward on 8 Trainium2 NeuronCores.

y = activation_quant(x) @ weight_quant(w).T

Strategy (column-parallel / tensor-parallel on out_features):
  - each core holds the full x and a 1/8 shard of w along out_features
  - weight scale (mean|w|) computed on-device with a 1-scalar AllReduce
  - activation quant: per-token int8 values, stored exactly in bf16
  - weight quant: ternary {-1,0,1} in bf16, transposed on the PE (via
    identity matmul) and converted to fp8e4 (still exact) for SBUF
    residency -- no DRAM round-trip for either quantized tensor's
    weights, and half the SBUF footprint vs bf16
  - matmul: bf16 stationary (x^T tile) x fp8 moving (w^T) -> exact fp32
    integer accumulation in PSUM at full PE rate
  - per-token output scale (1/act_scale * mean|w|) applied during PSUM
    drain; the weight-scale factor is folded into the drain scale so the
    activation-quant stream never waits on the AllReduce
  - host concatenates the 8 output shards along out_features

Schedule: (A) w pass-1 |w| sums stream in; (B) 1-scalar AllReduce with
a few token tiles quantizing during the wait; (C) w pass-2 requant
groups of 4 w tiles, and after each group the matmuls of the first two
token blocks run for that group's n-columns (nn-major prologue), so the
PE pipelines weight production with real matmul work; (D) steady-state:
per 512-row block, transpose-read the quantized activations and run the
k-outer matmul stream, with x-quant running a couple of blocks ahead.
The x-quant DRAM round-trip (bf16) remains: its DMA transpose-read is
the cheapest way to get K onto partitions for the stationary operand.
"""

import numpy as np
from contextlib import ExitStack

import concourse.bass as bass
import concourse.tile as tile
from concourse import bacc, mybir
from concourse import masks
from concourse.bass import ts, ds
from concourse.bass_utils import run_bass_kernel_spmd

F32 = mybir.dt.float32
BF16 = mybir.dt.bfloat16
FP8 = mybir.dt.float8e4
AX = mybir.AxisListType
OP = mybir.AluOpType
ACTF = mybir.ActivationFunctionType

# 1.5 * 2^23: adding then subtracting performs round-to-nearest-even at
# integer granularity in fp32 (matches jnp.round for |v| <= 2^21).
MAGIC = 12582912.0
EPS = 1e-5
P = 128


class _Builder:
    def __init__(self, nc, M, K, N, count_total, n_cores,
                 skip_quant=False, skip_matmul=False, wqt_dtype=FP8):
        self.nc = nc
        self.M, self.K, self.N = M, K, N
        self.count_total = count_total
        self.n_cores = n_cores
        self.skip_quant = skip_quant
        self.skip_matmul = skip_matmul
        self.wqt_dtype = wqt_dtype
        self.KT = K // P          # 32 k-slices
        self.MT = M // P          # 64 token tiles
        self.NCH = 512
        self.NT = N // self.NCH   # 4 psum chunks
        self.MB = 512             # token rows per transpose block
        self.MSUB = self.MB // P  # 4 tiles per block
        self.MBT = M // self.MB   # 16 blocks
        self.XH = K // 2          # half-row granularity for streaming
        self.WT = N // P          # 16 weight tiles
        self.PROB = 2             # prologue spans this many token blocks
        # x-quant emission runs this many tiles ahead of the matmul stream
        self.QAHEAD = 8
        self._xpend = {}
        self._next_load = 0
        self._next_comp = 0

    def build(self, reps=1):
        nc = self.nc
        self.x = nc.dram_tensor("x", [self.M, self.K], F32,
                                kind="ExternalInput").ap()
        self.w = nc.dram_tensor("w", [self.N, self.K], F32,
                                kind="ExternalInput").ap()
        self.y = nc.dram_tensor("y", [self.M, self.N], F32,
                                kind="ExternalOutput").ap()

        with tile.TileContext(nc) as tc, ExitStack() as ctx:
            self.tc = tc
            dram = ctx.enter_context(
                tc.tile_pool(name="dram", bufs=1, space="DRAM"))
            self.xq_dram = dram.tile([self.M, self.K], BF16)
            self.cc_in = dram.tile([1, 1], F32)
            self.cc_out = dram.tile([1, 1], F32)

            const = ctx.enter_context(tc.tile_pool(name="const", bufs=1))
            self.onesf = const.tile([P, P], F32)
            nc.vector.memset(self.onesf[:], 1.0)
            self.ident = const.tile([P, P], BF16)
            masks.make_identity(nc, self.ident[:])
            self.alphas = const.tile([P, self.MT], F32)   # 1/act_scale
            self.alphaf = const.tile([P, self.MT], F32)   # * s_w
            self.wsums = const.tile([P, self.WT], F32)
            self.s_w_eff = const.tile([P, 1], F32)
            self.s_inv = const.tile([P, 1], F32)

            self.stat = ctx.enter_context(tc.tile_pool(name="stat", bufs=8))

            if self.skip_quant:
                nc.vector.memset(self.alphaf[:], 1.0)
                seed = const.tile([P, 64], BF16)
                nc.vector.memset(seed[:], 1.0)
                nc.gpsimd.dma_start(self.xq_dram[0:P, 0:64], seed[:])

            for _ in range(reps):
                self._xpend = {}
                self._next_load = 0
                self._next_comp = 0
                self.emit_once(ctx)
        return nc

    def emit_once(self, ctx):
        with ExitStack() as mctx:
            tc = self.tc
            if not self.skip_matmul:
                self.wqt_pool = mctx.enter_context(
                    tc.tile_pool(name="wqt", bufs=1))
                self.wqt = self.wqt_pool.tile(
                    [P, self.KT, self.N], self.wqt_dtype)
            if not self.skip_quant:
                self.xin = mctx.enter_context(
                    tc.tile_pool(name="x_in", bufs=6))
                self.xqs = mctx.enter_context(
                    tc.tile_pool(name="xq_st", bufs=2))

            if self.skip_quant:
                # timing-ablation: matmul stream only, off seeded DRAM
                self.open_matmul_pools(mctx)
                for mb in range(self.MBT):
                    xqtt = self.emit_block_transposes(mb)
                    for ms in range(self.MSUB):
                        self.emit_tile_matmul(
                            mb * self.MSUB + ms, xqtt, ms, seeded=True)
            elif self.skip_matmul:
                with ExitStack() as wctx:
                    self.win = wctx.enter_context(
                        tc.tile_pool(name="w_in", bufs=4))
                    self.wqs = wctx.enter_context(
                        tc.tile_pool(name="wq_st", bufs=2))
                    scr = wctx.enter_context(
                        tc.tile_pool(name="wscr", bufs=2))
                    for t in range(self.WT):
                        self.emit_wreduce(t)
                    with ExitStack() as sctx:
                        self.pss = sctx.enter_context(
                            tc.tile_pool(name="pssmall", bufs=1,
                                         space="PSUM"))
                        self.emit_scale_pre()
                        self.emit_scale_post()
                    self.psc = wctx.enter_context(
                        tc.tile_pool(name="psc", bufs=8, space="PSUM"))
                    self._psc_pending = []
                    for t in range(self.WT):
                        self.emit_wpass2_tile(t, scratch=scr)
                    self.flush_psc()
                for _ in range(self.MT):
                    self.emit_xq_step()
            else:
                self.emit_main(mctx)

    # ---------------- weight pass 1: |w| partial sums ---------------------
    def emit_wreduce(self, t):
        nc = self.nc
        accs = []
        for h in range(2):
            wt = self.win.tile([P, self.XH], F32, tag="w", name="wt")
            nc.scalar.dma_start(
                wt[:], self.w[ts(t, P), ds(h * self.XH, self.XH)])
            a = self.stat.tile([P, 1], F32, tag=f"wr{h}", name="a")
            nc.vector.tensor_reduce(
                a[:], wt[:], axis=AX.X, op=OP.add,
                apply_absolute_value=True)
            accs.append(a)
        nc.vector.tensor_tensor(
            self.wsums[:, ds(t, 1)], accs[0][:], accs[1][:], OP.add)

    # ---------------- global scale + AllReduce ----------------------------
    def emit_scale_pre(self):
        nc = self.nc
        wpart = self.stat.tile([P, 1], F32, tag="wpart", name="wpart")
        nc.vector.tensor_reduce(
            wpart[:], self.wsums[:], axis=AX.X, op=OP.add)
        ps1 = self.pss.tile([1, 1], F32, tag="small", name="ps1")
        # ones[128,1].T @ wpart[128,1] -> total |w| sum on this core
        nc.tensor.matmul(
            ps1[:], self.onesf[:, 0:1], wpart[:], start=True, stop=True)
        ccs = self.stat.tile([1, 1], F32, tag="ccs", name="ccs")
        nc.vector.tensor_copy(ccs[:], ps1[:])
        nc.sync.dma_start(self.cc_in[:], ccs[:])
        if self.n_cores > 1:
            nc.gpsimd.collective_compute(
                "AllReduce", OP.add,
                replica_groups=[list(range(self.n_cores))],
                ins=[self.cc_in.opt()], outs=[self.cc_out.opt()])
        else:
            nc.gpsimd.dma_start(self.cc_out[:], self.cc_in[:])

    def emit_scale_post(self):
        nc = self.nc
        ccb = self.stat.tile([1, 1], F32, tag="ccb", name="ccb")
        nc.sync.dma_start(ccb[:], self.cc_out[:])
        ps2 = self.pss.tile([P, 1], F32, tag="small", name="ps2")
        # ones[1,128].T @ val[1,1] -> broadcast scalar to 128 partitions
        nc.tensor.matmul(
            ps2[:], self.onesf[0:1, :], ccb[:], start=True, stop=True)
        # mean = sum/count; count is a power of two so multiply is exact
        assert self.count_total & (self.count_total - 1) == 0
        s_w = self.stat.tile([P, 1], F32, tag="s_w", name="s_w")
        nc.vector.tensor_scalar(
            s_w[:], ps2[:], 1.0 / float(self.count_total), EPS,
            OP.mult, OP.max)
        nc.vector.reciprocal(self.s_inv[:], s_w[:])
        nc.vector.reciprocal(self.s_w_eff[:], self.s_inv[:])

    # ---------------- weight pass 2: requant + PE transpose + fp8 ---------
    PSC_BUFS = 8

    def flush_psc(self, room=0):
        nc = self.nc
        while len(self._psc_pending) > room:
            pst, dst = self._psc_pending.pop(0)
            nc.scalar.activation(dst, pst[:, :, :], ACTF.Copy)

    def emit_wpass2_tile(self, t, scratch=None):
        """Requant one w tile (per half) and PE-transpose it straight into
        the fp8 wqt resident tensor. PSUM->wqt copies are deferred until
        their psc slot is about to be reused."""
        nc = self.nc
        for h in range(2):
            wt = self.win.tile([P, self.XH], F32, tag="w", name="wt")
            nc.scalar.dma_start(
                wt[:], self.w[ts(t, P), ds(h * self.XH, self.XH)])
            # fused quant on DVE: round(w*s_inv), clip to [-1,1], to bf16
            nc.vector.tensor_scalar(
                wt[:], wt[:], self.s_inv[:, 0:1], MAGIC, OP.mult, OP.add)
            nc.vector.tensor_scalar(
                wt[:], wt[:], -MAGIC, 1.0, OP.add, OP.min)
            wq = self.wqs.tile([P, self.XH], BF16, tag="wq", name="wq")
            nc.vector.tensor_scalar(wq[:], wt[:], -1.0, None, OP.max)
            for kk in range(self.KT // 8):
                self.flush_psc(self.PSC_BUFS - 1)
                pst = self.psc.tile([P, 4, P], BF16, tag="psc", name="pst")
                for j in range(4):
                    nc.tensor.transpose(
                        pst[:, j, :], wq[:, ds((4 * kk + j) * P, P)],
                        self.ident[:])
                k0 = h * (self.KT // 2) + 4 * kk
                if scratch is not None:
                    dst = scratch.tile([P, 4, P], self.wqt_dtype,
                                       tag="scr", name="scr")[:, :, :]
                else:
                    dst = self.wqt[:, ds(k0, 4), ds(t * P, P)]
                self._psc_pending.append((pst, dst))

    # ---------------- activation quant, software-pipelined ----------------
    def emit_xload(self, mt):
        nc = self.nc
        halves = []
        for h in range(2):
            xt = self.xin.tile([P, self.XH], F32, tag="x", name="xt")
            nc.sync.dma_start(
                xt[:], self.x[ts(mt, P), ds(h * self.XH, self.XH)])
            halves.append(xt)
        self._xpend[mt] = halves

    def emit_xcompute(self, mt):
        nc = self.nc
        stat = self.stat
        xs = self._xpend.pop(mt)
        am0 = stat.tile([P, 1], F32, tag="am0", name="am0")
        am1 = stat.tile([P, 1], F32, tag="am1", name="am1")
        nc.vector.tensor_reduce(
            am0[:], xs[0][:], axis=AX.X, op=OP.max,
            apply_absolute_value=True)
        nc.vector.tensor_reduce(
            am1[:], xs[1][:], axis=AX.X, op=OP.max,
            apply_absolute_value=True)
        ame = stat.tile([P, 1], F32, tag="ame", name="ame")
        nc.vector.tensor_tensor(ame[:], am0[:], am1[:], OP.max)
        amc = stat.tile([P, 1], F32, tag="amc", name="amc")
        nc.vector.tensor_scalar(amc[:], ame[:], EPS, None, OP.max)
        rec = stat.tile([P, 1], F32, tag="rec", name="rec")
        nc.vector.reciprocal(rec[:], amc[:])
        scale = stat.tile([P, 1], F32, tag="scale", name="scale")
        nc.vector.tensor_scalar(scale[:], rec[:], 127.0, None, OP.mult)
        # alphas[:, mt] = 1/scale (s_w factor folded in at drain time)
        nc.vector.reciprocal(self.alphas[:, ds(mt, 1)], scale[:])
        for h in range(2):
            # fused quant: (x * scale + MAGIC) - MAGIC, rounded, in bf16
            nc.vector.tensor_scalar(
                xs[h][:], xs[h][:], scale[:, 0:1], MAGIC, OP.mult, OP.add)
            xq = self.xqs.tile([P, self.XH], BF16, tag="xq", name="xq")
            nc.vector.tensor_scalar(xq[:], xs[h][:], -MAGIC, None, OP.add)
            nc.sync.dma_start(
                self.xq_dram[ts(mt, P), ds(h * self.XH, self.XH)], xq[:])

    def emit_xq_step(self):
        # one tile of quant progress: prefetch next loads, compute current
        if self._next_load < self.MT:
            self.emit_xload(self._next_load)
            self._next_load += 1
        if self._next_comp < self._next_load:
            self.emit_xcompute(self._next_comp)
            self._next_comp += 1

    # ---------------- matmul stream ---------------------------------------
    def emit_block_transposes(self, mb):
        nc = self.nc
        xqtt = self.xqt.tile([P, self.KT, self.MB], BF16, tag="xqt",
                             name="xqtt")
        for k in range(self.KT):
            nc.sync.dma_start_transpose(
                xqtt[:, k, :],
                self.xq_dram[ds(mb * self.MB, self.MB), ds(k * P, P)])
        return xqtt

    def _alphaf(self, mt):
        nc = self.nc
        nc.scalar.activation(
            self.alphaf[:, ds(mt, 1)], self.alphas[:, ds(mt, 1)],
            ACTF.Copy, scale=self.s_w_eff[:, 0:1])

    def _drain(self, mt, nn, psum):
        nc = self.nc
        ot = self.outp.tile([P, self.NCH], F32, tag="o", name="ot")
        nc.scalar.activation(
            ot[:], psum[:], ACTF.Copy, scale=self.alphaf[:, ds(mt, 1)])
        nc.gpsimd.dma_start(
            self.y[ts(mt, P), ds(nn * self.NCH, self.NCH)], ot[:])

    def emit_tile_matmul(self, mt, xqtt, ms, seeded=False):
        nc = self.nc
        if not seeded:
            self._alphaf(mt)
        psums = [
            self.pspool.tile([P, self.NCH], F32, tag=f"ps{nn}",
                             name=f"ps{nn}")
            for nn in range(self.NT)
        ]
        for k in range(self.KT):
            lhs = xqtt[:, k, ds(ms * P, P)]
            first, last = (k == 0), (k == self.KT - 1)
            for nn in range(self.NT):
                nc.tensor.matmul(
                    psums[nn][:], lhs,
                    self.wqt[:, k, ds(nn * self.NCH, self.NCH)],
                    start=first, stop=last)
        for nn in range(self.NT):
            self._drain(mt, nn, psums[nn])

    def open_matmul_pools(self, mctx):
        tc = self.tc
        self.xqt = mctx.enter_context(tc.tile_pool(name="xqt", bufs=2))
        self.pspool = mctx.enter_context(
            tc.tile_pool(name="ps", bufs=2, space="PSUM"))
        self.outp = mctx.enter_context(tc.tile_pool(name="out", bufs=2))

    # ---------------- full pipeline ---------------------------------------
    def emit_main(self, mctx):
        tc = self.tc
        nc = self.nc
        NPRO = self.PROB * self.MSUB  # prologue token tiles
        with ExitStack() as wctx:
            self.win = wctx.enter_context(tc.tile_pool(name="w_in", bufs=4))
            self.wqs = wctx.enter_context(tc.tile_pool(name="wq_st", bufs=2))
            # Phase A: w pass-1 |w| sums (the AllReduce gates everything
            # w-side downstream, so it streams first).
            for t in range(self.WT):
                self.emit_wreduce(t)
            # Phase B: AllReduce; 4 token tiles quantize during the wait.
            with ExitStack() as sctx:
                self.pss = sctx.enter_context(
                    tc.tile_pool(name="pssmall", bufs=1, space="PSUM"))
                self.emit_scale_pre()
                for _ in range(4):
                    self.emit_xq_step()
                self.emit_scale_post()
            # Phase C: w pass-2 burst (requant + PE transpose + fp8), with
            # 8 more token tiles interleaved so the ring keeps filling.
            self.psc = wctx.enter_context(
                tc.tile_pool(name="psc", bufs=self.PSC_BUFS, space="PSUM"))
            self._psc_pending = []
            for t in range(self.WT):
                self.emit_wpass2_tile(t)
                if t % 2 == 1:
                    self.emit_xq_step()
            self.flush_psc()
        # Phase D: the matmul pools open now, reusing the freed w-pool
        # SBUF/PSUM space. Prologue: the first 2 blocks run nn-major so
        # the PE consumes wqt n-column groups in production order.
        self.open_matmul_pools(mctx)
        xb = [self.emit_block_transposes(b) for b in range(self.PROB)]
        for nn in range(self.NT):
            for mt in range(NPRO):
                mb, ms = divmod(mt, self.MSUB)
                if nn == 0:
                    self._alphaf(mt)
                psum = self.pspool.tile(
                    [P, self.NCH], F32, tag=f"ps{nn}", name=f"ps{nn}")
                for k in range(self.KT):
                    nc.tensor.matmul(
                        psum[:], xb[mb][:, k, ds(ms * P, P)],
                        self.wqt[:, k, ds(nn * self.NCH, self.NCH)],
                        start=(k == 0), stop=(k == self.KT - 1))
                self._drain(mt, nn, psum)
        for mb in range(self.PROB, self.MBT):
            # pre-emit enough quant at the prologue exit that the load
            # stream is not frozen behind T(b2) waiting on its ring slot
            want = min(self.MT,
                       max((self.PROB + 4) * self.MSUB,
                           mb * self.MSUB + self.QAHEAD))
            while self._next_comp < want:
                self.emit_xq_step()
            xqtt = self.emit_block_transposes(mb)
            for ms in range(self.MSUB):
                self.emit_tile_matmul(mb * self.MSUB + ms, xqtt, ms)


def build_bitlinear(nc, M, K, N, count_total, n_cores, reps=1,
                    skip_quant=False, skip_matmul=False, wqt_dtype=FP8):
    return _Builder(nc, M, K, N, count_total, n_cores,
                    skip_quant=skip_quant, skip_matmul=skip_matmul,
                    wqt_dtype=wqt_dtype).build(reps)


def dedupe_ldweights(nc):
    """Drop InstLdweights that reload the exact weights already resident in
    the PE array. The tile pipeline splits every bf16 matmul into
    LDWEIGHTS+MATMUL(ldweights=False); consecutive matmuls sharing one
    stationary operand (the nn loop) therefore reload it redundantly.
    The stationary operand persists across non-self-loading matmuls, so a
    sync-free LDW identical to the previous one is a no-op. Run after
    nc.compile() so all semaphore passes have finalized sync_info."""
    removed = 0
    for fn in nc.m.functions:
        for blk in fn.blocks:
            last_sig = None
            keep = []
            for inst in blk.instructions:
                if isinstance(inst, mybir.InstLdweights):
                    a = inst.ins[0]
                    sig = (getattr(a, "memref", None), a.offset, str(a.ap),
                           str(a.dtype), str(inst.perf_mode),
                           str(inst.tile_position), str(inst.is_transpose))
                    si = inst.sync_info
                    clean = si is None or (not si.on_wait and not si.on_update)
                    if sig == last_sig and clean and sig[0] is not None:
                        removed += 1
                        continue
                    last_sig = sig
                elif isinstance(inst, mybir.InstMatmult):
                    if inst.ldweights is not False:
                        last_sig = None  # self-loading matmul clobbers weights
                elif getattr(inst, "engine", None) == mybir.EngineType.PE:
                    if inst.is_executable():
                        last_sig = None  # unknown PE instruction: be safe
                keep.append(inst)
            if len(keep) != len(blk.instructions):
                blk.instructions = keep
    return removed


# ----------------------------------------------------------------------------
# Host-side entry point
# ----------------------------------------------------------------------------

_FULL = dict(B=4, S=2048, K=4096, N_TOTAL=16384, N_CORES=8)
_CACHE = {}


def _make_nc(reps=1, skip_quant=False, skip_matmul=False):
    cfg = _FULL
    M = cfg["B"] * cfg["S"]
    n_shard = cfg["N_TOTAL"] // cfg["N_CORES"]
    nc = bacc.Bacc(
        "TRN2",
        target_bir_lowering=False,
        debug=False,
        num_devices=cfg["N_CORES"],
    )
    build_bitlinear(
        nc, M=M, K=cfg["K"], N=n_shard,
        count_total=cfg["N_TOTAL"] * cfg["K"],
        n_cores=cfg["N_CORES"],
        reps=reps, skip_quant=skip_quant, skip_matmul=skip_matmul,
    )
    nc.compile()
    dedupe_ldweights(nc)
    from concourse.bass_interp import get_hw_module
    nc.m = get_hw_module(nc.m)
    return nc


def _get_compiled():
    if "nc" not in _CACHE:
        _CACHE["nc"] = _make_nc()
    return _CACHE["nc"]


def kernel(x: np.ndarray, weight: np.ndarray, _trace: bool = False):
    cfg = _FULL
    M = cfg["B"] * cfg["S"]
    n_shard = cfg["N_TOTAL"] // cfg["N_CORES"]
    nc = _get_compiled()

    x2 = np.ascontiguousarray(np.asarray(x, dtype=np.float32).reshape(M, cfg["K"]))
    wf = np.asarray(weight, dtype=np.float32)
    in_maps = [
        {"x": x2, "w": np.ascontiguousarray(wf[i * n_shard:(i + 1) * n_shard])}
        for i in range(cfg["N_CORES"])
    ]
    res = run_bass_kernel_spmd(
        nc, in_maps, list(range(cfg["N_CORES"])), trace=_trace)
    _CACHE["last_result"] = res
    yfull = np.concatenate(
        [res.results[i]["y"] for i in range(cfg["N_CORES"])], axis=1)
    return yfull.reshape(cfg["B"], cfg["S"], cfg["N_TOTAL"])
